# revision 23
# baseline (speedup 1.0000x reference)
"""Spatial-reduction attention (PVT-style) on 8 Trainium2 NeuronCores.

Sharding: core = (batch b, token half). Each core computes the full attention
output for its 2048 query tokens; the SR conv + LN + KV path (1024 reduced
tokens) is replicated across the 2 cores of a batch, so the kernel needs no
cross-core communication.

Each core receives xt = x[b].T with columns permuted tap-major:
col' = (2i+j)*1024 + 32h + w for original token t = 128h + 64i + 2w + j
(i,j = conv tap, h,w = reduced-image position; for the half=1 core h is
rolled by 16 so its own query half sits at r<512 of every tap block). This
makes every conv-tap matmul operand a contiguous slice (the HW BIR verifier
allows only one free dimension on matmul APs), removes the host tap gather,
and only permutes the reduced K/V token order and the query column order —
both of which attention is invariant to; the host unpermutes the output
columns.

Stage C/D runs as a 128-slot software pipeline over (query-block, head):
scores (8 matmuls, K=64) -> one [128,1024] exp on ACT -> AV (8 matmuls,
lagged one slot) -> per-half softmax normalize (DVE reciprocal + gpsimd
partition_broadcast + DVE multiply) -> proj deferred two slots. PSUM is
exactly 8 banks: scores ring 4, matmul ring 2, AV accumulators 2.
"""
import numpy as np
import ml_dtypes

import concourse.bass as bass
import concourse.tile as tile
from concourse import library_config
from concourse import mybir
from concourse.bass_utils import run_bass_kernel_spmd

import bass_rust

F32 = mybir.dt.float32
BF16 = mybir.dt.bfloat16

B, N, C = 4, 4096, 512
H, HD = 8, 64
SIDE = 64           # sqrt(N)
RS = 32             # reduced side
NP = RS * RS        # 1024 reduced tokens
HALF = N // 2       # 2048 query tokens per core
QB = 128            # query block
NQB = HALF // QB    # 16
LN_EPS = 1e-5
SCALE = HD ** -0.5

# weight order inside "wrest": wkT, wvT, wqT, wpT
W_K, W_V, W_Q, W_P = 0, 1, 2, 3


# --------------------------------------------------------------------------
# walrus workaround: this container's neuronx-cc rejects >1 sync-wait per
# instruction; hoist extras onto same-engine NoOps (program order preserved).
def _fixup_sync_waits(nc):
    fixed = 0
    for fn in nc.m.functions:
        for bb in fn.blocks:
            out = []
            changed = False
            for inst in bb.instructions:
                si = getattr(inst, "sync_info", None)
                waits = list(si.on_wait) if (si and si.on_wait) else []
                if len(waits) > 1:
                    for w in waits[:-1]:
                        nop = mybir.InstNoOp(
                            name=f"I-waitfix-{nc.next_id()}", ins=[], outs=[])
                        nop.engine = inst.engine
                        nop.sync_info = bass_rust.SyncInfo(
                            on_wait=[w], on_update=[])
                        out.append(nop)
                    si.on_wait = waits[-1:]
                    fixed += len(waits) - 1
                    changed = True
                out.append(inst)
            if changed:
                bb.instructions = out
    return fixed


# --------------------------------------------------------------------------
def build_nc(fixup=True, reps=1, trace_sim=False, stages="ABCD",
             loop_reps=None, use_bias=True, use_pbias=True):
    nc = bass.Bass()
    dp = nc.declare_dram_parameter

    xt_e = dp("xt", [C, N], BF16, isOutput=False)      # rolled x[b].T
    wtap_e = dp("wtap", [4 * C, C], BF16, isOutput=False)
    wrest_e = dp("wrest", [4 * C, C], BF16, isOutput=False)
    srb_e = dp("srb", [1, C], BF16, isOutput=False)
    bk_e = dp("bkrow", [1, C], BF16, isOutput=False)
    bv_e = dp("bvrow", [1, C], BF16, isOutput=False)
    bp_e = dp("bprow", [1, C], BF16, isOutput=False)
    id_e = dp("ident", [128, 128], BF16, isOutput=False)
    yt_e = dp("ytb", [C, HALF], BF16, isOutput=True)

    Exp = mybir.ActivationFunctionType.Exp
    Ln = mybir.ActivationFunctionType.Ln

    with tile.TileContext(nc, trace_sim=trace_sim) as tc:
        with tc.tile_pool(name="pp", bufs=1) as pp, \
             tc.tile_pool(name="ps", bufs=1, space="PSUM") as ps, \
             tc.tile_pool(name="pC", bufs=1) as pC, \
             tc.tile_pool(name="dr", bufs=3, space="DRAM") as drp:
            # ------- persistent weights & constants (loaded once) -------
            wc = pp.tile([128, 16, C], BF16)
            nc.gpsimd.dma_start(
                out=wc, in_=wrest_e.rearrange("(w p) n -> p w n", p=128))
            wtap = pp.tile([128, 16, C], BF16)
            nc.gpsimd.dma_start(
                out=wtap, in_=wtap_e.rearrange("(w p) n -> p w n", p=128))
            ident = pp.tile([128, 128], BF16)
            nc.sync.dma_start(out=ident, in_=id_e[:])
            srb = pp.tile([1, C], BF16)
            nc.sync.dma_start(out=srb, in_=srb_e[:])
            bkrow = pp.tile([1, C], BF16)
            nc.sync.dma_start(out=bkrow, in_=bk_e[:])
            bvrow = pp.tile([1, C], BF16)
            nc.sync.dma_start(out=bvrow, in_=bv_e[:])
            bprow = pp.tile([1, C], BF16)
            nc.sync.dma_start(out=bprow, in_=bp_e[:])
            ones1 = pp.tile([1, 128], BF16)
            nc.vector.memset(ones1, 1.0)
            onesN = pp.tile([1, C], BF16)
            nc.vector.memset(onesN, 1.0)
            eps_t = pp.tile([128, 1], F32)
            nc.vector.memset(eps_t, LN_EPS)

            # ------- persistent activation tiles -------
            xt = pp.tile([128, 4, N], BF16)
            xcT = pp.tile([128, 4, NP], BF16)
            kT = pp.tile([128, 4, NP], BF16)
            vaug = pp.tile([128, 8, H, HD + 1], BF16)
            nc.vector.memset(vaug[:, :, :, HD:HD + 1], 1.0)
            qT = pp.tile([128, 4, HALF], BF16)
            outA = pp.tile([128, 4, HALF], BF16)

            xt_r = xt_e.rearrange("(c p) n -> p c n", p=128)
            yt_r = yt_e.rearrange("(m p) q -> p m q", p=128)

            loop_cm = tc.For_i(0, loop_reps, 1) if loop_reps else None
            if loop_cm:
                loop_cm.__enter__()
            for _rep in range(reps):
                # per-iteration activation load: 8 contiguous 512-col chunks
                # (1KB DMA lines); even chunks first so conv ct0-3 (which
                # read cols [0,512) of every tap block) can start early
                for blk, sh in [(0, 0), (1, 0), (2, 0), (3, 0),
                                (0, 1), (1, 1), (2, 1), (3, 1)]:
                    c0 = blk * 1024 + sh * 512
                    nc.sync.dma_start(out=xt[:, :, c0:c0 + 512],
                                      in_=xt_r[:, :, c0:c0 + 512])

                # ---------------- stage A: conv + LN + kv ----------------
                if "A" in stages:
                    for ct in range(8):
                        pc = ps.tile([128, C], F32, tag="mm", name="conv",
                                     bufs=2)
                        first = True
                        for t in range(4):
                            for cc in range(4):
                                last = (t == 3 and cc == 3)
                                nc.tensor.matmul(
                                    pc,
                                    xt[:, cc, t * 1024 + ct * 128:
                                       t * 1024 + (ct + 1) * 128],
                                    wtap[:, t * 4 + cc, :],
                                    start=first,
                                    stop=(last and not use_bias))
                                first = False
                        if use_bias:
                            nc.tensor.matmul(pc, ones1, srb,
                                             start=False, stop=True)
                        lnw = pp.tile([128, 10], F32, tag="lnw", name="lnw",
                                      bufs=3)
                        stats = lnw[:, 0:6]
                        mv = lnw[:, 6:8]
                        sd = lnw[:, 8:9]
                        rstd = lnw[:, 9:10]
                        nc.vector.bn_stats(out=stats, in_=pc)
                        nc.vector.bn_aggr(out=mv, in_=stats)
                        nc.scalar.activation(out=sd, in_=mv[:, 1:2], func=Ln,
                                             bias=eps_t, scale=1.0)
                        nc.scalar.activation(out=rstd, in_=sd, func=Exp,
                                             scale=-0.5)
                        xc = pp.tile([128, C], BF16, tag="xc", name="xc",
                                     bufs=3)
                        nc.vector.tensor_scalar(
                            out=xc, in0=pc, scalar1=mv[:, 0:1], scalar2=rstd,
                            op0=mybir.AluOpType.subtract,
                            op1=mybir.AluOpType.mult)
                        pt4 = ps.tile([128, 4, 128], BF16, tag="s", name="tp",
                                      bufs=2)
                        for cc in range(4):
                            nc.tensor.transpose(
                                pt4[:, cc, :], xc[:, cc * 128:(cc + 1) * 128],
                                ident)
                        nc.vector.tensor_copy(
                            xcT[:, :, ct * 128:(ct + 1) * 128], pt4)

                    for m in range(4):
                        for hf in range(2):
                            pk = ps.tile([128, 512], F32, tag="mm", name="kv",
                                         bufs=2)
                            for cc in range(4):
                                nc.tensor.matmul(
                                    pk,
                                    wc[:, W_K * 4 + cc, m * 128:(m + 1) * 128],
                                    xcT[:, cc, hf * 512:(hf + 1) * 512],
                                    start=(cc == 0),
                                    stop=(cc == 3 and not use_bias))
                            if use_bias:
                                nc.tensor.matmul(
                                    pk, bkrow[:, m * 128:(m + 1) * 128],
                                    onesN[:, 0:512], start=False, stop=True)
                            nc.vector.tensor_copy(
                                kT[:, m, hf * 512:(hf + 1) * 512], pk)
                    for ct in range(8):
                        pv = ps.tile([128, 512], F32, tag="mm", name="kv",
                                     bufs=2)
                        for cc in range(4):
                            nc.tensor.matmul(
                                pv, xcT[:, cc, ct * 128:(ct + 1) * 128],
                                wc[:, W_V * 4 + cc, :],
                                start=(cc == 0),
                                stop=(cc == 3 and not use_bias))
                        if use_bias:
                            nc.tensor.matmul(pv, ones1, bvrow,
                                             start=False, stop=True)
                        nc.vector.tensor_copy(
                            vaug[:, ct, :, 0:HD],
                            pv.rearrange("p (h d) -> p h d", h=H))

                # ---------------- stage B: q (ns=0; rest interleaved) ----
                def emit_q(m, ns):
                    # own query half = first 512 columns of tap block ns
                    pq = ps.tile([128, 512], F32, tag="mm", name="q", bufs=2)
                    for cc in range(4):
                        nc.tensor.matmul(
                            pq, wc[:, W_Q * 4 + cc, m * 128:(m + 1) * 128],
                            xt[:, cc, ns * 1024:ns * 1024 + 512],
                            start=(cc == 0), stop=(cc == 3))
                    nc.vector.tensor_copy(
                        qT[:, m, ns * 512:(ns + 1) * 512], pq)

                if "B" in stages:
                    for m in range(4):
                        emit_q(m, 0)

                # -------- stage C/D: 128-slot (qb, h) pipeline --------
                if "C" in stages:
                    pav_t = [None, None]

                    def scores_exp(qb, h):
                        m, p0 = h // 2, 64 * (h % 2)
                        pss = ps.tile([128, 8, 128], F32, tag="s", name="s",
                                      bufs=2)
                        for kt in range(8):
                            nc.tensor.matmul(
                                pss[:, kt, :],
                                kT[p0:p0 + 64, m, kt * 128:(kt + 1) * 128],
                                qT[p0:p0 + 64, m, qb * 128:(qb + 1) * 128],
                                start=True, stop=True)
                        pb = pC.tile([128, 8, 128], BF16, tag="pb", name="pb",
                                     bufs=3)
                        nc.scalar.activation(out=pb, in_=pss, func=Exp)
                        return pb

                    def emit_av(qb, h, pb):
                        g, hh = h // 4, h % 4
                        if hh == 0:
                            pav_t[g] = ps.tile([65, 4, 128], F32,
                                               tag=f"av{g}", name=f"av{g}",
                                               bufs=1)
                        pav = pav_t[g]
                        for kt in range(8):
                            nc.tensor.matmul(
                                pav[:, hh, :], vaug[:, kt, h, :],
                                pb[:, kt, :], start=(kt == 0), stop=(kt == 7))

                    # finalize: recips + value copies accumulate per group
                    # of 4 qbs; ONE DRAM-roundtrip broadcast per group (small
                    # dependent DMAs on the in-order queue are very slow on
                    # HW), then 32 normalize multiplies and 4 projs.
                    acc_t = [None, None, None]   # rqacc, otqacc, yacc

                    def finalize_half(qb, g):
                        pav = pav_t[g]
                        k8 = (qb % 4) * 2 + g
                        if k8 == 0:
                            acc_t[0] = pC.tile([1, 8, 512], BF16, tag="rqa",
                                               name="rqa", bufs=2)
                            acc_t[1] = pC.tile([64, 8, 512], BF16, tag="otq",
                                               name="otq", bufs=2)
                        with nc.allow_low_precision("softmax denom in bf16"):
                            nc.vector.reciprocal(
                                acc_t[0][:, k8, :].rearrange(
                                    "p (a b) -> p a b", a=4),
                                pav[64:65, :, :])
                        nc.vector.tensor_copy(
                            acc_t[1][:, k8, :].rearrange(
                                "p (a b) -> p a b", a=4),
                            pav[0:64, :, :])

                    def emit_group_norm(gi):
                        rqacc, otqacc = acc_t[0], acc_t[1]
                        dq = drp.tile([1, 4096], BF16, tag="dq", name="dq")
                        nc.sync.dma_start(
                            out=dq, in_=rqacc.rearrange("p a b -> p (a b)"))
                        rb = pC.tile([64, 8, 512], BF16, tag="rb", name="rb",
                                     bufs=2)
                        nc.sync.dma_start(
                            out=rb.rearrange("p a b -> p (a b)"),
                            in_=dq.partition_broadcast(64))
                        for k8 in range(8):
                            qb, g = gi * 4 + k8 // 2, k8 % 2
                            for hh in range(4):
                                h2 = g * 4 + hh
                                m2, p0b = h2 // 2, 64 * (h2 % 2)
                                nc.vector.tensor_mul(
                                    outA[p0b:p0b + 64, m2,
                                         qb * 128:(qb + 1) * 128],
                                    otqacc[:, k8, hh * 128:(hh + 1) * 128],
                                    rb[:, k8, hh * 128:(hh + 1) * 128])

                    def emit_proj(qb):
                        py = ps.tile([128, 4, 128], F32, tag="mm", name="p",
                                     bufs=2)
                        for mo in range(4):
                            for cc in range(4):
                                nc.tensor.matmul(
                                    py[:, mo, :],
                                    wc[:, W_P * 4 + cc, mo * 128:(mo + 1) * 128],
                                    outA[:, cc, qb * 128:(qb + 1) * 128],
                                    start=(cc == 0),
                                    stop=(cc == 3 and not use_pbias))
                            if use_pbias:
                                nc.tensor.matmul(
                                    py[:, mo, :],
                                    bprow[:, mo * 128:(mo + 1) * 128],
                                    ones1[:, 0:128], start=False, stop=True)
                        if qb % 4 == 0:
                            acc_t[2] = pC.tile([128, 4, 512], BF16, tag="ys",
                                               name="ys", bufs=2)
                        nc.vector.tensor_copy(
                            acc_t[2][:, :, (qb % 4) * 128:(qb % 4 + 1) * 128],
                            py)
                        if qb % 4 == 3:
                            gi = qb // 4
                            nc.sync.dma_start(
                                out=yt_r[:, :, gi * 512:(gi + 1) * 512],
                                in_=acc_t[2])

                    pending = {}
                    due = {}
                    for s in range(NQB * 8 + 13):
                        # scores+exp first in each slot so ACT never waits on
                        # the slot's filler matmuls (q lookahead, proj)
                        if s < NQB * 8:
                            qb, h = divmod(s, 8)
                            pending[s] = (qb, h, scores_exp(qb, h))
                        if s >= 1 and (s - 1) in pending:
                            qb1, h1, pb1 = pending.pop(s - 1)
                            emit_av(qb1, h1, pb1)
                            if h1 == 3:
                                finalize_half(qb1, 0)
                            elif h1 == 7:
                                finalize_half(qb1, 1)
                                if qb1 % 4 == 3:
                                    gi = qb1 // 4
                                    emit_group_norm(gi)
                                    if "D" in stages:
                                        for k in range(4):
                                            due[s + 2 + 2 * k] = gi * 4 + k
                        if s < NQB * 8:
                            if h == 1 and "B" in stages and qb < 12:
                                emit_q(qb % 4, qb // 4 + 1)
                        if s in due:
                            emit_proj(due.pop(s))

            if loop_cm:
                loop_cm.__exit__(None, None, None)

    if fixup:
        _fixup_sync_waits(nc)
    return nc


_NC = {}


def _get_nc(key):
    if key not in _NC:
        _NC[key] = build_nc(use_bias=key[0], use_pbias=key[1])
    return _NC[key]


def _host_prep(x, Wq, Wkv, sr_w, sr_b, ln_g, ln_b, proj_w, proj_b):
    bf = ml_dtypes.bfloat16
    f32 = np.float32
    x = np.asarray(x, f32)
    Wq = np.asarray(Wq, f32)
    Wkv = np.asarray(Wkv, f32)
    sr_w = np.asarray(sr_w, f32)
    sr_b = np.asarray(sr_b, f32)
    ln_g = np.asarray(ln_g, f32)
    ln_b = np.asarray(ln_b, f32)
    proj_w = np.asarray(proj_w, f32)
    proj_b = np.asarray(proj_b, f32)

    Wk, Wv = Wkv[:C], Wkv[C:]
    wtap = np.concatenate(
        [sr_w[:, :, 0, 0].T, sr_w[:, :, 0, 1].T,
         sr_w[:, :, 1, 0].T, sr_w[:, :, 1, 1].T], axis=0)
    wrest = np.concatenate(
        [(Wk * ln_g[None, :]).T, (Wv * ln_g[None, :]).T,
         (Wq * SCALE).T, proj_w.T], axis=0)
    weights = {
        "wtap": np.ascontiguousarray(wtap).astype(bf),
        "wrest": np.ascontiguousarray(wrest).astype(bf),
        "srb": sr_b[None, :].astype(bf),
        "bkrow": (Wk @ ln_b)[None, :].astype(bf),
        "bvrow": (Wv @ ln_b)[None, :].astype(bf),
        "bprow": proj_b[None, :].astype(bf),
        "ident": np.eye(128, dtype=f32).astype(bf),
    }

    in_maps = []
    xt_cache = {}
    for core in range(8):
        b, half = core // 2, core % 2
        m = dict(weights)
        if (b, half) not in xt_cache:
            xT = np.ascontiguousarray(x[b].T).astype(bf)   # [C, N]
            xt_cache[(b, half)] = np.ascontiguousarray(
                xT[:, _tap_perm(half)])
        m["xt"] = xt_cache[(b, half)]
        in_maps.append(m)
    return in_maps


def _tap_perm(half):
    """Permuted xt column -> original token. col' = blk*1024 + 32h_loc + w,
    token = 128h + 64i + 2w + j with (i,j) = tap block, h = h_loc rolled."""
    cols = np.empty(N, np.int64)
    h_loc = np.repeat(np.arange(32), 32)
    w = np.tile(np.arange(32), 32)
    h = (h_loc + 16 * half) % 32
    for blk in range(4):
        i, j = blk // 2, blk % 2
        cols[blk * 1024:(blk + 1) * 1024] = 128 * h + 64 * i + 2 * w + j
    return cols


def _qmap(half):
    """ytb column -> original token index (own half)."""
    k = np.arange(HALF)
    ns, r = k // 512, k % 512
    h = r // 32 + 16 * half
    w = r % 32
    i, j = ns // 2, ns % 2
    return 128 * h + 64 * i + 2 * w + j


def kernel(**inputs):
    in_maps = _host_prep(**inputs)
    use_bias = bool(np.any(np.asarray(inputs["sr_b"]))
                    or np.any(np.asarray(inputs["ln_b"])))
    use_pbias = bool(np.any(np.asarray(inputs["proj_b"])))
    nc = _get_nc((use_bias, use_pbias))
    res = run_bass_kernel_spmd(nc, in_maps, core_ids=list(range(8)))
    out = np.empty((B, N, C), np.float32)
    for core in range(8):
        b, half = core // 2, core % 2
        out[b, _qmap(half), :] = res.results[core]["ytb"].T.astype(np.float32)
    return out
